# revision 13
# baseline (speedup 1.0000x reference)
"""Causal single-head attention on 8 Trainium2 NeuronCores (batch-parallel).

Problem (nn_Head): x[32,1024,256] f32, Wk/Wq/Wv[64,256] f32.
  q/k/v = x @ W.T ; wei = softmax(causal(q @ k.T / 8)) ; out = wei @ v.

Sharding: B=32 split 4-per-core across 8 cores; weights replicated.

Per-core kernel (per batch item, all matmuls bf16 with f32 PSUM accum):
  - host pre-transposes x to x^T [C,T] (bf16) so projections use W as the
    stationary operand with big streaming dims.
  - fused weights [Wq^T|Wk^T] ("A": q^T in partitions 0-63, k^T in 64-127)
    and [Wk^T|Wq^T] ("B") give both q^T and k^T on partitions 0-63, which the
    score matmuls need (lhsT = k^T chunk, rhs = q^T span, contraction=HS=64).
  - scores are computed transposed, wei^T[s,t], only for causal-valid blocks;
    exp runs on ScalarE straight out of PSUM with scale=1/8 folded in
    (no max-subtraction: |scores| <~ 4 so exp cannot overflow); the single
    diagonal 128x128 chunk per s-tile is masked multiplicatively post-exp.
  - v is computed in natural [t,h] layout with an appended ones column, so
    the PV matmul (lhsT = exp(wei^T) chunk, rhs = v_aug) yields both the
    unnormalized out[t,h] and the softmax denominator in column HS.
  - normalize via per-partition reciprocal-scale, store natural [t,h] f32.
"""

import numpy as np
import ml_dtypes

B, T, C, HS = 32, 1024, 256, 64
NCORES = 8
BPC = B // NCORES  # batch items per core
P = 128            # partitions / row-tile
NT = T // P        # 8 row tiles per item
CO = C // P        # 2 contraction chunks for projections
TCH = 512          # matmul free-dim chunk (one PSUM bank of f32)
NTC = T // TCH     # 2

_cached = {}


def _build():
    import concourse.tile as tile
    from concourse import bacc, mybir

    bf16 = mybir.dt.bfloat16
    f32 = mybir.dt.float32
    Exp = mybir.ActivationFunctionType.Exp

    nc = bacc.Bacc(
        "TRN2",
        target_bir_lowering=False,
        debug=False,
        num_devices=NCORES,
    )

    xT = nc.dram_tensor("xT", [BPC, C, T], bf16, kind="ExternalInput")
    wA = nc.dram_tensor("wA", [C, 2 * HS], bf16, kind="ExternalInput")  # [WqT|WkT]
    wB = nc.dram_tensor("wB", [C, 2 * HS], bf16, kind="ExternalInput")  # [WkT|WqT]
    wV = nc.dram_tensor("wV", [C, HS], bf16, kind="ExternalInput")      # WvT
    mask = nc.dram_tensor("mask", [P, P], bf16, kind="ExternalInput")   # triu(ones)
    out = nc.dram_tensor("out", [BPC, T, HS], f32, kind="ExternalOutput")

    with tile.TileContext(nc) as tc:
        with (
            tc.tile_pool(name="consts", bufs=1) as consts,
            tc.tile_pool(name="xin", bufs=4) as xin,
            tc.tile_pool(name="proj", bufs=2) as projp,
            tc.tile_pool(name="expw", bufs=2) as expwp,
            tc.tile_pool(name="outp", bufs=3) as outp,
            tc.tile_pool(name="ps_sc", bufs=2, space="PSUM") as ps_sc,
            tc.tile_pool(name="ps_sm", bufs=2, space="PSUM") as ps_sm,
        ):
            wA_sb = consts.tile([P, CO, 2 * HS], bf16, tag="wA")
            nc.gpsimd.dma_start(wA_sb, wA.rearrange("(co p) m -> p co m", p=P))
            wB_sb = consts.tile([P, CO, 2 * HS], bf16, tag="wB")
            nc.gpsimd.dma_start(wB_sb, wB.rearrange("(co p) m -> p co m", p=P))
            wV_sb = consts.tile([P, CO, HS], bf16, tag="wV")
            nc.gpsimd.dma_start(wV_sb, wV.rearrange("(co p) m -> p co m", p=P))
            mask_sb = consts.tile([P, P], bf16, tag="mask")
            nc.gpsimd.dma_start(mask_sb, mask[:, :])

            for it in range(BPC):
                # split the x^T load across two DMA queues (one co-half each)
                xT_sb = xin.tile([P, CO, T], bf16, tag="xT")
                xT_r = xT[it].rearrange("(co p) t -> p co t", p=P)
                nc.sync.dma_start(xT_sb[:, 0:1, :], xT_r[:, 0:1, :])
                nc.gpsimd.dma_start(xT_sb[:, 1:2, :], xT_r[:, 1:2, :])

                # fused q/k projections: A = (q^T top, k^T bottom), B swapped
                A_sb = projp.tile([P, T], bf16, tag="Asb")
                B_sb = projp.tile([P, T], bf16, tag="Bsb")
                for w_sb, o_sb in ((wA_sb, A_sb), (wB_sb, B_sb)):
                    # co outer so each weight chunk is loaded once per item
                    pss = [
                        ps_sm.tile([P, TCH], f32, tag="proj", name=f"pss{h}")
                        for h in range(NTC)
                    ]
                    for co in range(CO):
                        for h in range(NTC):
                            nc.tensor.matmul(
                                pss[h],
                                w_sb[:, co, :],
                                xT_sb[:, co, h * TCH:(h + 1) * TCH],
                                start=(co == 0),
                                stop=(co == CO - 1),
                            )
                    for h in range(NTC):
                        nc.vector.tensor_copy(
                            o_sb[:, h * TCH:(h + 1) * TCH], pss[h]
                        )

                # v projection in natural [t,h] layout + ones column for the
                # softmax denominator; all 8 t-tiles share one PSUM bank so a
                # single cast evacuates them
                vaug = projp.tile([P, NT, HS + 1], bf16, tag="vaug")
                nc.vector.memset(vaug[:, :, HS:HS + 1], 1.0)
                psv = ps_sm.tile([P, NT, HS], f32, tag="proj")
                for ti in range(NT):
                    for co in range(CO):
                        nc.tensor.matmul(
                            psv[:, ti, :],
                            xT_sb[:, co, ti * P:(ti + 1) * P],
                            wV_sb[:, co, :],
                            start=(co == 0),
                            stop=(co == CO - 1),
                        )
                nc.vector.tensor_copy(vaug[:, :, 0:HS], psv)

                # scores wei^T[s,t]; even s-tiles contract on PE rows 0-63
                # (k^T/q^T from B/A top halves), odd s-tiles concurrently on
                # rows 64-127 (A/B bottom halves) — 2x score throughput
                expw = [None] * NT
                for sp in range(NT // 2):
                    pss2 = []
                    for j in range(2):  # j=0 -> rows 0-63, j=1 -> rows 64-127
                        si = 2 * sp + j
                        t_lo = si * P
                        tjd = t_lo // TCH
                        base = tjd * TCH
                        kT = B_sb if j == 0 else A_sb
                        qT = A_sb if j == 0 else B_sb
                        lo, hi = (0, HS) if j == 0 else (HS, P)
                        ps = ps_sc.tile([P, 2 * TCH], f32, tag="sc",
                                        name=f"sc{si}")
                        for tj in range(tjd, NTC):
                            t0 = max(tj * TCH, t_lo)
                            t1 = (tj + 1) * TCH
                            nc.tensor.matmul(
                                ps[:, t0 - base:t1 - base],
                                kT[lo:hi, si * P:(si + 1) * P],
                                qT[lo:hi, t0:t1],
                                start=True,
                                stop=True,
                            )
                        pss2.append((si, t_lo, base, ps))
                    for si, t_lo, base, ps in pss2:
                        ew = expwp.tile([P, T - t_lo], bf16, tag=f"ew{si}",
                                        name=f"ew{si}")
                        nc.scalar.activation(
                            ew, ps[:, t_lo - base:T - base], Exp, scale=0.125
                        )
                        # mask the diagonal 128-wide chunk (keep s <= t)
                        nc.vector.tensor_mul(ew[:, 0:P], ew[:, 0:P], mask_sb)
                        expw[si] = ew

                # PV: natural out[t,h]; col HS carries the softmax denominator.
                # 4 t-tiles share one PSUM bank so normalization is batched:
                # one reciprocal + one broadcast-multiply per half.
                o_sb = outp.tile([P, NT, HS], f32, tag="osb")
                for half in range(2):
                    po = ps_sm.tile([P, 4, HS + 1], f32, tag="pv")
                    for tii in range(4):
                        ti = half * 4 + tii
                        for si in range(ti + 1):
                            nc.tensor.matmul(
                                po[:, tii, :],
                                expw[si][:, (ti - si) * P:(ti - si + 1) * P],
                                vaug[:, si, :],
                                start=(si == 0),
                                stop=(si == ti),
                            )
                    r = outp.tile([P, 4], f32, tag="recip")
                    nc.vector.reciprocal(r, po[:, :, HS])
                    nc.vector.tensor_tensor(
                        o_sb[:, half * 4:(half + 1) * 4, :],
                        po[:, :, 0:HS],
                        r[:, :, None].to_broadcast([P, 4, HS]),
                        mybir.AluOpType.mult,
                    )
                nc.sync.dma_start(
                    out[it].rearrange("(ti p) h -> p ti h", p=P), o_sb
                )

    nc.compile()
    return nc


def _get_nc():
    nc = _cached.get("nc")
    if nc is None:
        nc = _build()
        _cached["nc"] = nc
    return nc


def _in_maps(x, Wk, Wq, Wv):
    bf = ml_dtypes.bfloat16
    x = np.asarray(x, dtype=np.float32)
    Wk = np.asarray(Wk, dtype=np.float32)
    Wq = np.asarray(Wq, dtype=np.float32)
    Wv = np.asarray(Wv, dtype=np.float32)
    wA = np.ascontiguousarray(np.concatenate([Wq.T, Wk.T], axis=1)).astype(bf)
    wB = np.ascontiguousarray(np.concatenate([Wk.T, Wq.T], axis=1)).astype(bf)
    wV = np.ascontiguousarray(Wv.T).astype(bf)
    m = np.triu(np.ones((P, P), dtype=np.float32)).astype(bf)
    maps = []
    for c in range(NCORES):
        xs = x[c * BPC:(c + 1) * BPC]
        xsT = np.ascontiguousarray(xs.transpose(0, 2, 1)).astype(bf)
        maps.append({"xT": xsT, "wA": wA, "wB": wB, "wV": wV, "mask": m})
    return maps


def _run(x, Wk, Wq, Wv, **spmd_kwargs):
    from concourse.bass_utils import run_bass_kernel_spmd

    nc = _get_nc()
    res = run_bass_kernel_spmd(
        nc, _in_maps(x, Wk, Wq, Wv), core_ids=list(range(NCORES)), **spmd_kwargs
    )
    full = np.concatenate([r["out"] for r in res.results], axis=0)
    return full, res


def kernel(x, Wk, Wq, Wv):
    full, _ = _run(x, Wk, Wq, Wv)
    return full


# revision 14
# speedup vs baseline: 1.0010x; 1.0010x over previous
"""Causal single-head attention on 8 Trainium2 NeuronCores (batch-parallel).

Problem (nn_Head): x[32,1024,256] f32, Wk/Wq/Wv[64,256] f32.
  q/k/v = x @ W.T ; wei = softmax(causal(q @ k.T / 8)) ; out = wei @ v.

Sharding: B=32 split 4-per-core across 8 cores; weights replicated.

Per-core kernel (per batch item, all matmuls bf16 with f32 PSUM accum):
  - host pre-transposes x to x^T [C,T] (bf16) so projections use W as the
    stationary operand with big streaming dims.
  - fused weights [Wq^T|Wk^T] ("A": q^T in partitions 0-63, k^T in 64-127)
    and [Wk^T|Wq^T] ("B") give both q^T and k^T on partitions 0-63, which the
    score matmuls need (lhsT = k^T chunk, rhs = q^T span, contraction=HS=64).
  - scores are computed transposed, wei^T[s,t], only for causal-valid blocks;
    exp runs on ScalarE straight out of PSUM with scale=1/8 folded in
    (no max-subtraction: |scores| <~ 4 so exp cannot overflow); the single
    diagonal 128x128 chunk per s-tile is masked multiplicatively post-exp.
  - v is computed in natural [t,h] layout with an appended ones column, so
    the PV matmul (lhsT = exp(wei^T) chunk, rhs = v_aug) yields both the
    unnormalized out[t,h] and the softmax denominator in column HS.
  - normalize via per-partition reciprocal-scale, store natural [t,h] f32.
"""

import numpy as np
import ml_dtypes

B, T, C, HS = 32, 1024, 256, 64
NCORES = 8
BPC = B // NCORES  # batch items per core
P = 128            # partitions / row-tile
NT = T // P        # 8 row tiles per item
CO = C // P        # 2 contraction chunks for projections
TCH = 512          # matmul free-dim chunk (one PSUM bank of f32)
NTC = T // TCH     # 2

_cached = {}


def _build():
    import concourse.tile as tile
    from concourse import bacc, mybir

    bf16 = mybir.dt.bfloat16
    f32 = mybir.dt.float32
    Exp = mybir.ActivationFunctionType.Exp

    nc = bacc.Bacc(
        "TRN2",
        target_bir_lowering=False,
        debug=False,
        num_devices=NCORES,
    )

    xT = nc.dram_tensor("xT", [BPC, C, T], bf16, kind="ExternalInput")
    wA = nc.dram_tensor("wA", [C, 2 * HS], bf16, kind="ExternalInput")  # [WqT|WkT]
    wB = nc.dram_tensor("wB", [C, 2 * HS], bf16, kind="ExternalInput")  # [WkT|WqT]
    wV = nc.dram_tensor("wV", [C, HS], bf16, kind="ExternalInput")      # WvT
    mask = nc.dram_tensor("mask", [P, P], bf16, kind="ExternalInput")   # triu(ones)
    out = nc.dram_tensor("out", [BPC, T, HS], f32, kind="ExternalOutput")

    with tile.TileContext(nc) as tc:
        with (
            tc.tile_pool(name="consts", bufs=1) as consts,
            tc.tile_pool(name="xin", bufs=4) as xin,
            tc.tile_pool(name="proj", bufs=2) as projp,
            tc.tile_pool(name="expw", bufs=2) as expwp,
            tc.tile_pool(name="outp", bufs=3) as outp,
            tc.tile_pool(name="ps_sc", bufs=2, space="PSUM") as ps_sc,
            tc.tile_pool(name="ps_sm", bufs=2, space="PSUM") as ps_sm,
        ):
            wA_sb = consts.tile([P, CO, 2 * HS], bf16, tag="wA")
            nc.gpsimd.dma_start(wA_sb, wA.rearrange("(co p) m -> p co m", p=P))
            wB_sb = consts.tile([P, CO, 2 * HS], bf16, tag="wB")
            nc.gpsimd.dma_start(wB_sb, wB.rearrange("(co p) m -> p co m", p=P))
            wV_sb = consts.tile([P, CO, HS], bf16, tag="wV")
            nc.gpsimd.dma_start(wV_sb, wV.rearrange("(co p) m -> p co m", p=P))
            mask_sb = consts.tile([P, P], bf16, tag="mask")
            nc.gpsimd.dma_start(mask_sb, mask[:, :])

            for it in range(BPC):
                # split the x^T load across two DMA queues (one co-half each)
                xT_sb = xin.tile([P, CO, T], bf16, tag="xT")
                xT_r = xT[it].rearrange("(co p) t -> p co t", p=P)
                nc.sync.dma_start(xT_sb[:, 0:1, :], xT_r[:, 0:1, :])
                nc.gpsimd.dma_start(xT_sb[:, 1:2, :], xT_r[:, 1:2, :])

                # fused q/k projections: A = (q^T top, k^T bottom), B swapped
                A_sb = projp.tile([P, T], bf16, tag="Asb")
                B_sb = projp.tile([P, T], bf16, tag="Bsb")
                for w_sb, o_sb in ((wA_sb, A_sb), (wB_sb, B_sb)):
                    # co outer so each weight chunk is loaded once per item
                    pss = [
                        ps_sm.tile([P, TCH], f32, tag="proj", name=f"pss{h}")
                        for h in range(NTC)
                    ]
                    for co in range(CO):
                        for h in range(NTC):
                            nc.tensor.matmul(
                                pss[h],
                                w_sb[:, co, :],
                                xT_sb[:, co, h * TCH:(h + 1) * TCH],
                                start=(co == 0),
                                stop=(co == CO - 1),
                            )
                    for h in range(NTC):
                        nc.vector.tensor_copy(
                            o_sb[:, h * TCH:(h + 1) * TCH], pss[h]
                        )

                # v projection in natural [t,h] layout + ones column for the
                # softmax denominator; all 8 t-tiles share one PSUM bank so a
                # single cast evacuates them
                vaug = projp.tile([P, NT, HS + 1], bf16, tag="vaug")
                nc.vector.memset(vaug[:, :, HS:HS + 1], 1.0)
                psv = ps_sm.tile([P, NT, HS], f32, tag="proj")
                for ti in range(NT):
                    for co in range(CO):
                        nc.tensor.matmul(
                            psv[:, ti, :],
                            xT_sb[:, co, ti * P:(ti + 1) * P],
                            wV_sb[:, co, :],
                            start=(co == 0),
                            stop=(co == CO - 1),
                        )
                nc.vector.tensor_copy(vaug[:, :, 0:HS], psv)

                # scores wei^T[s,t] per s-tile; exp straight from PSUM
                expw = []
                for si in range(NT):
                    t_lo = si * P          # first causally-valid t for this s-tile
                    tjd = t_lo // TCH      # first t-chunk with valid columns
                    base = tjd * TCH       # psum tile covers t in [base, base+1024)
                    ps = ps_sc.tile([P, 2 * TCH], f32, tag="sc", name=f"sc{si}")
                    for tj in range(tjd, NTC):
                        t0 = max(tj * TCH, t_lo)
                        t1 = (tj + 1) * TCH
                        nc.tensor.matmul(
                            ps[:, t0 - base:t1 - base],
                            B_sb[0:HS, si * P:(si + 1) * P],  # k^T chunk
                            A_sb[0:HS, t0:t1],                # q^T span
                            start=True,
                            stop=True,
                        )
                    ew = expwp.tile([P, T - t_lo], bf16, tag=f"ew{si}",
                                    name=f"ew{si}")
                    nc.scalar.activation(
                        ew, ps[:, t_lo - base:T - base], Exp, scale=0.125
                    )
                    # mask the diagonal 128-wide chunk (keep s <= t)
                    nc.vector.tensor_mul(ew[:, 0:P], ew[:, 0:P], mask_sb)
                    expw.append(ew)

                # PV: natural out[t,h]; col HS carries the softmax denominator.
                # 4 t-tiles share one PSUM bank so normalization is batched:
                # one reciprocal + one broadcast-multiply per half.
                o_sb = outp.tile([P, NT, HS], f32, tag="osb")
                for half in range(2):
                    po = ps_sm.tile([P, 4, HS + 1], f32, tag="pv")
                    for tii in range(4):
                        ti = half * 4 + tii
                        for si in range(ti + 1):
                            nc.tensor.matmul(
                                po[:, tii, :],
                                expw[si][:, (ti - si) * P:(ti - si + 1) * P],
                                vaug[:, si, :],
                                start=(si == 0),
                                stop=(si == ti),
                            )
                    r = outp.tile([P, 4], f32, tag="recip")
                    nc.vector.reciprocal(r, po[:, :, HS])
                    nc.vector.tensor_tensor(
                        o_sb[:, half * 4:(half + 1) * 4, :],
                        po[:, :, 0:HS],
                        r[:, :, None].to_broadcast([P, 4, HS]),
                        mybir.AluOpType.mult,
                    )
                nc.sync.dma_start(
                    out[it].rearrange("(ti p) h -> p ti h", p=P), o_sb
                )

    nc.compile()
    return nc


def _get_nc():
    nc = _cached.get("nc")
    if nc is None:
        nc = _build()
        _cached["nc"] = nc
    return nc


def _in_maps(x, Wk, Wq, Wv):
    bf = ml_dtypes.bfloat16
    x = np.asarray(x, dtype=np.float32)
    Wk = np.asarray(Wk, dtype=np.float32)
    Wq = np.asarray(Wq, dtype=np.float32)
    Wv = np.asarray(Wv, dtype=np.float32)
    wA = np.ascontiguousarray(np.concatenate([Wq.T, Wk.T], axis=1)).astype(bf)
    wB = np.ascontiguousarray(np.concatenate([Wk.T, Wq.T], axis=1)).astype(bf)
    wV = np.ascontiguousarray(Wv.T).astype(bf)
    m = np.triu(np.ones((P, P), dtype=np.float32)).astype(bf)
    maps = []
    for c in range(NCORES):
        xs = x[c * BPC:(c + 1) * BPC]
        xsT = np.ascontiguousarray(xs.transpose(0, 2, 1)).astype(bf)
        maps.append({"xT": xsT, "wA": wA, "wB": wB, "wV": wV, "mask": m})
    return maps


def _run(x, Wk, Wq, Wv, **spmd_kwargs):
    from concourse.bass_utils import run_bass_kernel_spmd

    nc = _get_nc()
    res = run_bass_kernel_spmd(
        nc, _in_maps(x, Wk, Wq, Wv), core_ids=list(range(NCORES)), **spmd_kwargs
    )
    full = np.concatenate([r["out"] for r in res.results], axis=0)
    return full, res


def kernel(x, Wk, Wq, Wv):
    full, _ = _run(x, Wk, Wq, Wv)
    return full


# revision 15
# speedup vs baseline: 1.0216x; 1.0205x over previous
"""Causal single-head attention on 8 Trainium2 NeuronCores (batch-parallel).

Problem (nn_Head): x[32,1024,256] f32, Wk/Wq/Wv[64,256] f32.
  q/k/v = x @ W.T ; wei = softmax(causal(q @ k.T / 8)) ; out = wei @ v.

Sharding: B=32 split 4-per-core across 8 cores; weights replicated.

Per-core kernel (per batch item, all matmuls bf16 with f32 PSUM accum):
  - host pre-transposes x to x^T [C,T] (bf16) so projections use W as the
    stationary operand with big streaming dims.
  - fused weights [Wq^T|Wk^T] ("A": q^T in partitions 0-63, k^T in 64-127)
    and [Wk^T|Wq^T] ("B") give both q^T and k^T on partitions 0-63, which the
    score matmuls need (lhsT = k^T chunk, rhs = q^T span, contraction=HS=64).
  - scores are computed transposed, wei^T[s,t], only for causal-valid blocks;
    exp runs on ScalarE straight out of PSUM with scale=1/8 folded in
    (no max-subtraction: |scores| <~ 4 so exp cannot overflow); the single
    diagonal 128x128 chunk per s-tile is masked multiplicatively post-exp.
  - v is computed in natural [t,h] layout with an appended ones column, so
    the PV matmul (lhsT = exp(wei^T) chunk, rhs = v_aug) yields both the
    unnormalized out[t,h] and the softmax denominator in column HS.
  - normalize via per-partition reciprocal-scale, store natural [t,h] f32.
"""

import numpy as np
import ml_dtypes

B, T, C, HS = 32, 1024, 256, 64
NCORES = 8
BPC = B // NCORES  # batch items per core
P = 128            # partitions / row-tile
NT = T // P        # 8 row tiles per item
CO = C // P        # 2 contraction chunks for projections
TCH = 512          # matmul free-dim chunk (one PSUM bank of f32)
NTC = T // TCH     # 2

_cached = {}


def _build():
    import concourse.tile as tile
    from concourse import bacc, mybir

    bf16 = mybir.dt.bfloat16
    f32 = mybir.dt.float32
    Exp = mybir.ActivationFunctionType.Exp

    nc = bacc.Bacc(
        "TRN2",
        target_bir_lowering=False,
        debug=False,
        num_devices=NCORES,
    )

    xT = nc.dram_tensor("xT", [BPC, C, T], bf16, kind="ExternalInput")
    wA = nc.dram_tensor("wA", [C, 2 * HS], bf16, kind="ExternalInput")  # [WqT|WkT]
    wB = nc.dram_tensor("wB", [C, 2 * HS], bf16, kind="ExternalInput")  # [WkT|WqT]
    wV = nc.dram_tensor("wV", [C, HS], bf16, kind="ExternalInput")      # WvT
    mask = nc.dram_tensor("mask", [P, P], bf16, kind="ExternalInput")   # triu(ones)
    out = nc.dram_tensor("out", [BPC, T, HS], f32, kind="ExternalOutput")

    with tile.TileContext(nc) as tc:
        with (
            tc.tile_pool(name="consts", bufs=1) as consts,
            tc.tile_pool(name="xin", bufs=4) as xin,
            tc.tile_pool(name="proj", bufs=2) as projp,
            tc.tile_pool(name="expw", bufs=2) as expwp,
            tc.tile_pool(name="outp", bufs=3) as outp,
            tc.tile_pool(name="ps_sc", bufs=2, space="PSUM") as ps_sc,
            tc.tile_pool(name="ps_sm", bufs=2, space="PSUM") as ps_sm,
        ):
            xT_tiles = {}

            def load_xT(it):
                # split the x^T load across two DMA queues (one co-half each)
                t = xin.tile([P, CO, T], bf16, tag="xT", name=f"xT{it}")
                r = xT[it].rearrange("(co p) t -> p co t", p=P)
                nc.sync.dma_start(t[:, 0:1, :], r[:, 0:1, :])
                nc.gpsimd.dma_start(t[:, 1:2, :], r[:, 1:2, :])
                return t

            # item 0's x load goes out before any weight DMA (it gates the
            # first matmul); small weights ride the gpsimd/scalar queues
            xT_tiles[0] = load_xT(0)
            wA_sb = consts.tile([P, CO, 2 * HS], bf16, tag="wA")
            nc.gpsimd.dma_start(wA_sb, wA.rearrange("(co p) m -> p co m", p=P))
            wB_sb = consts.tile([P, CO, 2 * HS], bf16, tag="wB")
            nc.gpsimd.dma_start(wB_sb, wB.rearrange("(co p) m -> p co m", p=P))
            wV_sb = consts.tile([P, CO, HS], bf16, tag="wV")
            nc.scalar.dma_start(wV_sb, wV.rearrange("(co p) m -> p co m", p=P))
            mask_sb = consts.tile([P, P], bf16, tag="mask")
            nc.scalar.dma_start(mask_sb, mask[:, :])

            for it in range(BPC):
                xT_sb = xT_tiles.get(it)
                if xT_sb is None:
                    xT_sb = load_xT(it)

                # fused q/k projections: A = (q^T top, k^T bottom), B swapped
                A_sb = projp.tile([P, T], bf16, tag="Asb")
                B_sb = projp.tile([P, T], bf16, tag="Bsb")
                for w_sb, o_sb in ((wA_sb, A_sb), (wB_sb, B_sb)):
                    # co outer so each weight chunk is loaded once per item
                    pss = [
                        ps_sm.tile([P, TCH], f32, tag="proj", name=f"pss{h}")
                        for h in range(NTC)
                    ]
                    for co in range(CO):
                        for h in range(NTC):
                            nc.tensor.matmul(
                                pss[h],
                                w_sb[:, co, :],
                                xT_sb[:, co, h * TCH:(h + 1) * TCH],
                                start=(co == 0),
                                stop=(co == CO - 1),
                            )
                    for h in range(NTC):
                        nc.vector.tensor_copy(
                            o_sb[:, h * TCH:(h + 1) * TCH], pss[h]
                        )

                # v projection in natural [t,h] layout + ones column for the
                # softmax denominator; all 8 t-tiles share one PSUM bank so a
                # single cast evacuates them
                vaug = projp.tile([P, NT, HS + 1], bf16, tag="vaug")
                nc.vector.memset(vaug[:, :, HS:HS + 1], 1.0)
                psv = ps_sm.tile([P, NT, HS], f32, tag="proj")
                for ti in range(NT):
                    for co in range(CO):
                        nc.tensor.matmul(
                            psv[:, ti, :],
                            xT_sb[:, co, ti * P:(ti + 1) * P],
                            wV_sb[:, co, :],
                            start=(co == 0),
                            stop=(co == CO - 1),
                        )
                nc.vector.tensor_copy(vaug[:, :, 0:HS], psv)

                # scores wei^T[s,t] per s-tile; exp straight from PSUM
                expw = []
                for si in range(NT):
                    t_lo = si * P          # first causally-valid t for this s-tile
                    tjd = t_lo // TCH      # first t-chunk with valid columns
                    base = tjd * TCH       # psum tile covers t in [base, base+1024)
                    ps = ps_sc.tile([P, 2 * TCH], f32, tag="sc", name=f"sc{si}")
                    for tj in range(tjd, NTC):
                        t0 = max(tj * TCH, t_lo)
                        t1 = (tj + 1) * TCH
                        nc.tensor.matmul(
                            ps[:, t0 - base:t1 - base],
                            B_sb[0:HS, si * P:(si + 1) * P],  # k^T chunk
                            A_sb[0:HS, t0:t1],                # q^T span
                            start=True,
                            stop=True,
                        )
                    ew = expwp.tile([P, T - t_lo], bf16, tag=f"ew{si}",
                                    name=f"ew{si}")
                    nc.scalar.activation(
                        ew, ps[:, t_lo - base:T - base], Exp, scale=0.125
                    )
                    # mask the diagonal 128-wide chunk (keep s <= t)
                    nc.vector.tensor_mul(ew[:, 0:P], ew[:, 0:P], mask_sb)
                    expw.append(ew)

                # PV: natural out[t,h]; col HS carries the softmax denominator.
                # 4 t-tiles share one PSUM bank so normalization is batched:
                # one reciprocal + one broadcast-multiply per half.
                o_sb = outp.tile([P, NT, HS], f32, tag="osb")
                for half in range(2):
                    po = ps_sm.tile([P, 4, HS + 1], f32, tag="pv")
                    for tii in range(4):
                        ti = half * 4 + tii
                        for si in range(ti + 1):
                            nc.tensor.matmul(
                                po[:, tii, :],
                                expw[si][:, (ti - si) * P:(ti - si + 1) * P],
                                vaug[:, si, :],
                                start=(si == 0),
                                stop=(si == ti),
                            )
                    r = outp.tile([P, 4], f32, tag="recip")
                    nc.vector.reciprocal(r, po[:, :, HS])
                    nc.vector.tensor_tensor(
                        o_sb[:, half * 4:(half + 1) * 4, :],
                        po[:, :, 0:HS],
                        r[:, :, None].to_broadcast([P, 4, HS]),
                        mybir.AluOpType.mult,
                    )
                nc.sync.dma_start(
                    out[it].rearrange("(ti p) h -> p ti h", p=P), o_sb
                )

    nc.compile()
    return nc


def _get_nc():
    nc = _cached.get("nc")
    if nc is None:
        nc = _build()
        _cached["nc"] = nc
    return nc


def _in_maps(x, Wk, Wq, Wv):
    bf = ml_dtypes.bfloat16
    x = np.asarray(x, dtype=np.float32)
    Wk = np.asarray(Wk, dtype=np.float32)
    Wq = np.asarray(Wq, dtype=np.float32)
    Wv = np.asarray(Wv, dtype=np.float32)
    wA = np.ascontiguousarray(np.concatenate([Wq.T, Wk.T], axis=1)).astype(bf)
    wB = np.ascontiguousarray(np.concatenate([Wk.T, Wq.T], axis=1)).astype(bf)
    wV = np.ascontiguousarray(Wv.T).astype(bf)
    m = np.triu(np.ones((P, P), dtype=np.float32)).astype(bf)
    maps = []
    for c in range(NCORES):
        xs = x[c * BPC:(c + 1) * BPC]
        xsT = np.ascontiguousarray(xs.transpose(0, 2, 1)).astype(bf)
        maps.append({"xT": xsT, "wA": wA, "wB": wB, "wV": wV, "mask": m})
    return maps


def _run(x, Wk, Wq, Wv, **spmd_kwargs):
    from concourse.bass_utils import run_bass_kernel_spmd

    nc = _get_nc()
    res = run_bass_kernel_spmd(
        nc, _in_maps(x, Wk, Wq, Wv), core_ids=list(range(NCORES)), **spmd_kwargs
    )
    full = np.concatenate([r["out"] for r in res.results], axis=0)
    return full, res


def kernel(x, Wk, Wq, Wv):
    full, _ = _run(x, Wk, Wq, Wv)
    return full
